# revision 16
# baseline (speedup 1.0000x reference)
"""Trainium2 Bass kernel for nn_Decoder (attention decoder step + vocab projection).

Sharding (8 NeuronCores, single SPMD launch with one AllGather):
  Stage 1 (data-parallel over batch): each core gets B/8 = 64 batch rows and
    computes attention energies (fused DVE multiply-reduce), masked softmax,
    contexts, LSTM gates (PE matmuls, bf16) and the LSTM pointwise ops.
  AllGather: h_new^T (bf16, 128KB/core) across the 8 cores.
  Stage 2 (tensor-parallel over vocab): each core computes logits[:, v_slice]
    for the FULL batch against its 6250-row slice of w_proj (streamed from HBM
    as bf16, host-pretransposed to [E, V_slice]).

Host-side prep (layout/staging only): batch slicing, w_ih/w_hh/w_proj
transposition + bf16 cast, bias folding (b_ih + b_hh), padded-mask -> f32,
embedding row gather (pure indexing), output concatenation.

All softmax/attention arithmetic is fp32 on-device; PE matmuls run in bf16.
"""

from contextlib import ExitStack

import numpy as np
import ml_dtypes

import concourse.mybir as mybir
import concourse.tile as tile
from concourse import bacc, bass_utils
from concourse.masks import make_identity

# Problem constants (hardcoded per contest rules)
V = 50000
E = 1024
B = 512
ML = 50
NCORES = 8
BS = B // NCORES          # 64 batch rows per core
VS = V // NCORES          # 6250 vocab rows per core
KC = E // 128             # 8 contraction chunks
L2 = ML // 2              # 25 (two length-halves on partitions)
NT = (VS + 511) // 512    # 13 vocab n-tiles per core
G = 4 * E                 # 4096 gate columns

F32 = mybir.dt.float32
BF16 = mybir.dt.bfloat16
MULT = mybir.AluOpType.mult
ADD = mybir.AluOpType.add
AF = mybir.ActivationFunctionType

_CACHE = {}


def _body(nc, tc, d):
    with ExitStack() as top:
        const = top.enter_context(tc.tile_pool(name="const", bufs=1))
        sbm = top.enter_context(tc.tile_pool(name="sbm", bufs=1))
        pst = top.enter_context(tc.tile_pool(name="pst", bufs=2, space="PSUM"))
        psg = top.enter_context(tc.tile_pool(name="psg", bufs=2, space="PSUM"))
        pso = top.enter_context(tc.tile_pool(name="pso", bufs=4, space="PSUM"))
        dram = top.enter_context(tc.tile_pool(name="dram", bufs=1, space="DRAM"))

        identity = const.tile([128, 128], F32)
        make_identity(nc, identity[:])
        ones_bf = const.tile([1, 128], BF16)
        nc.vector.memset(ones_bf[:], 1.0)
        zbias = const.tile([128, 1], F32)
        nc.vector.memset(zbias[:], 0.0)

        h2 = sbm.tile([128, E], F32)
        nc.sync.dma_start(h2[0:64, :], d["h0s"])
        nc.sync.dma_start(h2[64:128, :], d["h0s"])
        words = sbm.tile([BS, E], F32)
        nc.sync.dma_start(words[:], d["words"])
        mask_sb = sbm.tile([BS, ML], F32)
        nc.sync.dma_start(mask_sb[:], d["mask"])
        ctxt = sbm.tile([BS, E], F32)
        h_new = sbm.tile([BS, E], F32)
        c_new = sbm.tile([BS, E], F32)

        # ================= stage 1: attention =================
        with ExitStack() as s1:
            encp = s1.enter_context(tc.tile_pool(name="encp", bufs=5))
            accp = s1.enter_context(tc.tile_pool(name="accp", bufs=2))
            tmp1 = s1.enter_context(tc.tile_pool(name="tmp1", bufs=1))

            enc_c = []
            for c in range(5):
                t = encp.tile([128, 5, E], F32, name=f"enc{c}", tag="enc")
                nc.sync.dma_start(t[:], d["enc"][:, c * 5:(c + 1) * 5, :])
                enc_c.append(t)
            enc_t = [enc_c[l2 // 5][:, l2 % 5, :] for l2 in range(L2)]

            energy2 = tmp1.tile([128, L2], F32)
            scr = tmp1.tile([128, E], F32)
            for l2 in range(L2):
                nc.vector.scalar_tensor_tensor(
                    out=scr[:], in0=enc_t[l2], scalar=1.0, in1=h2[:],
                    op0=MULT, op1=MULT, accum_out=energy2[:, l2:l2 + 1])

            energy = tmp1.tile([BS, ML], F32)
            nc.vector.tensor_copy(energy[:, 0:L2], energy2[0:64, :])
            nc.sync.dma_start(energy[:, L2:ML], energy2[64:128, :])

            # masked softmax (fp32; equals reference softmax+mask+renorm)
            rmax = tmp1.tile([BS, 1], F32)
            nc.vector.tensor_reduce(rmax[:], energy[:],
                                    axis=mybir.AxisListType.X,
                                    op=mybir.AluOpType.max)
            nrmax = tmp1.tile([BS, 1], F32)
            nc.vector.tensor_scalar_mul(nrmax[:], rmax[:], -1.0)
            expd = tmp1.tile([BS, ML], F32)
            nc.scalar.activation(expd[:], energy[:], AF.Exp,
                                 bias=nrmax[:], scale=1.0)
            masked = tmp1.tile([BS, ML], F32)
            nc.vector.tensor_mul(masked[:], expd[:], mask_sb[:])
            ssum = tmp1.tile([BS, 1], F32)
            nc.vector.tensor_reduce(ssum[:], masked[:],
                                    axis=mybir.AxisListType.X, op=ADD)
            rinv = tmp1.tile([BS, 1], F32)
            nc.vector.reciprocal(rinv[:], ssum[:])
            normed = tmp1.tile([BS, ML], F32)
            nc.vector.tensor_scalar_mul(normed[:], masked[:], rinv[:])

            normed2 = tmp1.tile([128, L2], F32)
            nc.vector.tensor_copy(normed2[0:64, :], normed[:, 0:L2])
            nc.sync.dma_start(normed2[64:128, :], normed[:, L2:ML])

            # contexts: chained multiply-accumulate (ping-pong)
            acc_prev = accp.tile([128, E], F32, name="acc0", tag="acc")
            nc.vector.tensor_scalar_mul(acc_prev[:], enc_t[0],
                                        normed2[:, 0:1])
            for l2 in range(1, L2):
                acc_cur = accp.tile([128, E], F32, name=f"acc{l2}", tag="acc")
                nc.vector.scalar_tensor_tensor(
                    out=acc_cur[:], in0=enc_t[l2],
                    scalar=normed2[:, l2:l2 + 1],
                    in1=acc_prev[:], op0=MULT, op1=ADD)
                acc_prev = acc_cur
            ctx_hi = tmp1.tile([BS, E], F32)
            nc.sync.dma_start(ctx_hi[:], acc_prev[64:128, :])
            nc.vector.tensor_add(ctxt[:], acc_prev[0:64, :], ctx_hi[:])

        # ================= gates + LSTM pointwise + stage 2 =================
        with ExitStack() as s2:
            gp = s2.enter_context(tc.tile_pool(name="gp", bufs=1))
            wgp = s2.enter_context(tc.tile_pool(name="wgp", bufs=3))
            sp = s2.enter_context(tc.tile_pool(name="sp", bufs=1))
            wtp = s2.enter_context(tc.tile_pool(name="wtp", bufs=3))
            outp = s2.enter_context(tc.tile_pool(name="outp", bufs=4))

            bproj_sb = sp.tile([1, VS], BF16)
            nc.sync.dma_start(bproj_sb[:], d["bproj"])
            biasg_sb = gp.tile([1, G], BF16)
            nc.sync.dma_start(biasg_sb[:], d["biasg"])

            # PE warmers: keep HAM at full clock through the DVE-only
            # attention window (results never read)
            for w in range(300):
                psd = psg.tile([BS, 512], F32, name=f"warm{w}", tag="gps")
                nc.tensor.matmul(psd[:], ones_bf[:, 0:BS], biasg_sb[:, 0:512],
                                 start=True, stop=True)

            wordsT = gp.tile([128, KC, BS], BF16)
            ctxT = gp.tile([128, KC, BS], BF16)
            for k in range(KC):
                pt = pst.tile([128, BS], F32, name=f"ptw{k}", tag="pt")
                nc.tensor.transpose(pt[:], words[:, k * 128:(k + 1) * 128],
                                    identity[0:BS, 0:BS])
                nc.scalar.copy(wordsT[:, k, :], pt[:])
                pt2 = pst.tile([128, BS], F32, name=f"ptc{k}", tag="pt")
                nc.tensor.transpose(pt2[:], ctxt[:, k * 128:(k + 1) * 128],
                                    identity[0:BS, 0:BS])
                nc.scalar.copy(ctxT[:, k, :], pt2[:])

            g_act = gp.tile([BS, 8, 512], F32)
            for q in range(4):
                wa = wgp.tile([128, KC, 1024], BF16, name=f"wih{q}", tag="wg")
                nc.sync.dma_start(
                    wa[:], d["wihT"][:, q * 1024:(q + 1) * 1024]
                    .rearrange("(k p) n -> p k n", p=128))
                wb = wgp.tile([128, KC, 1024], BF16, name=f"whh{q}", tag="wg")
                nc.sync.dma_start(
                    wb[:], d["whhT"][:, q * 1024:(q + 1) * 1024]
                    .rearrange("(k p) n -> p k n", p=128))
                for half in range(2):
                    n = q * 2 + half
                    hs = slice(half * 512, (half + 1) * 512)
                    ps = psg.tile([BS, 512], F32, name=f"gps{n}", tag="gps")
                    nc.tensor.matmul(ps[:], ones_bf[:, 0:BS],
                                     biasg_sb[:, n * 512:(n + 1) * 512],
                                     start=True, stop=False)
                    for k in range(KC):
                        nc.tensor.matmul(ps[:], wordsT[:, k, :], wa[:, k, hs],
                                         start=False, stop=False)
                    for k in range(KC):
                        nc.tensor.matmul(ps[:], ctxT[:, k, :], wb[:, k, hs],
                                         start=False, stop=(k == KC - 1))
                    func = AF.Tanh if n in (4, 5) else AF.Sigmoid
                    nc.scalar.activation(g_act[:, n, :], ps[:], func,
                                         bias=zbias[0:BS, :], scale=1.0)

            iv = g_act[:, 0:2, :]
            fv = g_act[:, 2:4, :]
            gv = g_act[:, 4:6, :]
            ov = g_act[:, 6:8, :]
            ctx_v = ctxt[:].rearrange("b (x y) -> b x y", x=2)
            ig = gp.tile([BS, 2, 512], F32)
            nc.vector.tensor_mul(ig[:], iv, gv)
            fc = gp.tile([BS, 2, 512], F32)
            nc.vector.tensor_mul(fc[:], fv, ctx_v)
            cn_v = c_new[:].rearrange("b (x y) -> b x y", x=2)
            nc.vector.tensor_add(cn_v, ig[:], fc[:])
            tanh_c = gp.tile([BS, E], F32)
            nc.scalar.activation(tanh_c[:], c_new[:], AF.Tanh,
                                 bias=zbias[0:BS, :], scale=1.0)
            th_v = tanh_c[:].rearrange("b (x y) -> b x y", x=2)
            hn_v = h_new[:].rearrange("b (x y) -> b x y", x=2)
            nc.vector.tensor_mul(hn_v, ov, th_v)
            nc.sync.dma_start(d["c_new_s"], c_new[:])
            nc.sync.dma_start(d["h_new_s"], h_new[:])

            # h_new^T (bf16) -> DRAM bounce
            hT = gp.tile([128, KC, BS], BF16)
            for k in range(KC):
                pt = pst.tile([128, BS], F32, name=f"pth{k}", tag="pt")
                nc.tensor.transpose(pt[:], h_new[:, k * 128:(k + 1) * 128],
                                    identity[0:BS, 0:BS])
                nc.scalar.copy(hT[:, k, :], pt[:])
            bounce = dram.tile([E, BS], BF16)
            nc.sync.dma_start(bounce[:].rearrange("(k p) b -> p k b", p=128),
                              hT[:])

            # stage-2 weight tiles: one 2MB DMA per 1024-col chunk (emitted
            # after the gates weight DMAs so the in-order queue cannot cycle)
            wt_wide = []
            for n2 in range((VS + 1023) // 1024):
                c0 = n2 * 1024
                cw = min(1024, VS - c0)
                wt = wtp.tile([128, KC, 1024], BF16, name=f"wt{n2}", tag="wt")
                nc.sync.dma_start(wt[:, :, 0:cw],
                                  d["wT"][:, c0:c0 + cw]
                                  .rearrange("(k p) n -> p k n", p=128))
                wt_wide.append(wt)

            gathered = dram.tile([NCORES, E, BS], BF16, addr_space="Shared")
            nc.gpsimd.collective_compute(
                "AllGather", mybir.AluOpType.bypass,
                replica_groups=[list(range(NCORES))],
                ins=[bounce[:].opt()], outs=[gathered[:].opt()])

            # PE warmers through the AllGather window
            for w in range(160):
                psd = psg.tile([BS, 512], F32, name=f"warm2_{w}", tag="gps")
                nc.tensor.matmul(psd[:], ones_bf[:, 0:BS], biasg_sb[:, 0:512],
                                 start=True, stop=True)

            # ================= stage 2: vocab projection =================
            HT = sp.tile([128, 4, KC, 128], BF16)
            for j in range(4):
                for k in range(KC):
                    src = gathered[2 * j:2 * j + 2, k * 128:(k + 1) * 128, :]
                    dst = HT[:, j, k, :].rearrange("p (c b) -> p c b", c=2)
                    nc.sync.dma_start(dst, src.rearrange("c e b -> e c b"))

            for n in range(NT):
                n0 = n * 512
                nw = min(512, VS - n0)
                wtile = wt_wide[n // 2]
                ws = slice((n % 2) * 512, (n % 2) * 512 + nw)
                for j in range(4):
                    ps = pso.tile([128, 512], F32, name=f"ops{n}_{j}", tag="ops")
                    nc.tensor.matmul(ps[:, 0:nw], ones_bf[:],
                                     bproj_sb[:, n0:n0 + nw],
                                     start=True, stop=False)
                    for k in range(KC):
                        nc.tensor.matmul(ps[:, 0:nw], HT[:, j, k, :],
                                         wtile[:, k, ws],
                                         start=False, stop=(k == KC - 1))
                    ot = outp.tile([128, 512], F32, name=f"ot{n}_{j}", tag="ot")
                    nc.scalar.copy(ot[:, 0:nw], ps[:, 0:nw])
                    nc.sync.dma_start(
                        d["logits_s"][j * 128:(j + 1) * 128, n0:n0 + nw],
                        ot[:, 0:nw])


def _build():
    nc = bacc.Bacc("TRN2", num_devices=NCORES)
    d = {}

    def inp(name, shape, dt):
        d[name] = nc.dram_tensor(name, shape, dt, kind="ExternalInput").ap()

    def outp(name, shape, dt):
        d[name] = nc.dram_tensor(name, shape, dt, kind="ExternalOutput").ap()

    inp("enc", [128, L2, E], F32)
    inp("h0s", [BS, E], F32)
    inp("words", [BS, E], F32)
    inp("mask", [BS, ML], F32)
    inp("wihT", [E, G], BF16)
    inp("whhT", [E, G], BF16)
    inp("biasg", [1, G], BF16)
    inp("wT", [E, VS], BF16)
    inp("bproj", [1, VS], BF16)
    outp("logits_s", [B, VS], F32)
    outp("h_new_s", [BS, E], F32)
    outp("c_new_s", [BS, E], F32)

    with tile.TileContext(nc) as tc:
        _body(nc, tc, d)
    nc.compile()
    return nc


def make_in_maps(target_sentences, encoder_outputs, h0, padded_positions,
                 emb, w_ih, w_hh, b_ih, b_hh, w_proj, b_proj):
    bf = ml_dtypes.bfloat16
    target_sentences = np.asarray(target_sentences)
    encoder_outputs = np.ascontiguousarray(
        np.asarray(encoder_outputs, np.float32))
    h00 = np.asarray(h0, np.float32)[0]
    padded = np.asarray(padded_positions)
    emb = np.asarray(emb, np.float32)
    w_proj = np.asarray(w_proj, np.float32)
    b_proj = np.asarray(b_proj, np.float32)

    words_all = emb[target_sentences]
    mask_all = 1.0 - padded.astype(np.float32)
    wihT = np.ascontiguousarray(np.asarray(w_ih, np.float32).T).astype(bf)
    whhT = np.ascontiguousarray(np.asarray(w_hh, np.float32).T).astype(bf)
    biasg = (np.asarray(b_ih, np.float32)
             + np.asarray(b_hh, np.float32))[None, :].astype(bf)
    wTf = np.ascontiguousarray(w_proj.T)

    in_maps = []
    for c in range(NCORES):
        bs = slice(c * BS, (c + 1) * BS)
        vs = slice(c * VS, (c + 1) * VS)
        enc_s = encoder_outputs[bs]
        encr = np.ascontiguousarray(
            enc_s.reshape(BS, 2, L2, E).transpose(1, 0, 2, 3).reshape(128, L2, E))
        in_maps.append({
            "enc": encr,
            "h0s": np.ascontiguousarray(h00[bs]),
            "words": np.ascontiguousarray(words_all[bs]),
            "mask": np.ascontiguousarray(mask_all[bs]),
            "wihT": wihT,
            "whhT": whhT,
            "biasg": biasg,
            "wT": np.ascontiguousarray(wTf[:, vs]).astype(bf),
            "bproj": np.ascontiguousarray(b_proj[vs])[None, :].astype(bf),
        })
    return in_maps


def kernel(target_sentences, encoder_outputs, h0, c0, padded_positions,
           max_len, emb, w_ih, w_hh, b_ih, b_hh, w_proj, b_proj):
    if "nc" not in _CACHE:
        _CACHE["nc"] = _build()
    nc = _CACHE["nc"]

    in_maps = make_in_maps(target_sentences, encoder_outputs, h0,
                           padded_positions, emb, w_ih, w_hh, b_ih, b_hh,
                           w_proj, b_proj)
    res = bass_utils.run_bass_kernel_spmd(nc, in_maps,
                                          core_ids=list(range(NCORES)))
    rs = res.results
    logits = np.concatenate([rs[c]["logits_s"] for c in range(NCORES)], axis=1)
    h_new = np.concatenate([rs[c]["h_new_s"] for c in range(NCORES)],
                           axis=0)[None]
    c_new = np.concatenate([rs[c]["c_new_s"] for c in range(NCORES)],
                           axis=0)[None]
    return logits, h_new, c_new


# revision 20
# speedup vs baseline: 1.3981x; 1.3981x over previous
"""Trainium2 Bass kernel for nn_Decoder (attention decoder step + vocab projection).

Sharding (8 NeuronCores, single SPMD launch with one AllGather):
  Stage 1 (data-parallel over batch): each core gets B/8 = 64 batch rows and
    computes attention energies (fused DVE multiply-reduce), masked softmax,
    contexts, LSTM gates (PE matmuls, bf16) and the LSTM pointwise ops.
  AllGather: h_new^T (bf16, 128KB/core) across the 8 cores.
  Stage 2 (tensor-parallel over vocab): each core computes logits[:, v_slice]
    for the FULL batch against its 6250-row slice of w_proj (streamed from HBM
    as bf16, host-pretransposed to [E, V_slice]).

Host-side prep (layout/staging only): batch slicing, w_ih/w_hh/w_proj
transposition + bf16 cast, bias folding (b_ih + b_hh), padded-mask -> f32,
embedding row gather (pure indexing), output concatenation.

All softmax/attention arithmetic is fp32 on-device; PE matmuls run in bf16.
"""

from contextlib import ExitStack

import numpy as np
import ml_dtypes

import concourse.mybir as mybir
import concourse.tile as tile
from concourse import bacc, bass_utils
from concourse.masks import make_identity

# Problem constants (hardcoded per contest rules)
V = 50000
E = 1024
B = 512
ML = 50
NCORES = 8
BS = B // NCORES          # 64 batch rows per core
VS = V // NCORES          # 6250 vocab rows per core
KC = E // 128             # 8 contraction chunks
L2 = ML // 2              # 25 (two length-halves on partitions)
NT = (VS + 511) // 512    # 13 vocab n-tiles per core
G = 4 * E                 # 4096 gate columns

F32 = mybir.dt.float32
BF16 = mybir.dt.bfloat16
MULT = mybir.AluOpType.mult
ADD = mybir.AluOpType.add
AF = mybir.ActivationFunctionType

_CACHE = {}


def _body(nc, tc, d):
    with ExitStack() as top:
        const = top.enter_context(tc.tile_pool(name="const", bufs=1))
        sbm = top.enter_context(tc.tile_pool(name="sbm", bufs=1))
        pst = top.enter_context(tc.tile_pool(name="pst", bufs=2, space="PSUM"))
        psg = top.enter_context(tc.tile_pool(name="psg", bufs=2, space="PSUM"))
        pso = top.enter_context(tc.tile_pool(name="pso", bufs=4, space="PSUM"))
        dram = top.enter_context(tc.tile_pool(name="dram", bufs=1, space="DRAM"))

        identity = const.tile([128, 128], F32)
        make_identity(nc, identity[:])
        ones_bf = const.tile([1, 128], BF16)
        nc.vector.memset(ones_bf[:], 1.0)
        zbias = const.tile([128, 1], F32)
        nc.vector.memset(zbias[:], 0.0)

        h2 = sbm.tile([128, E], F32)
        nc.sync.dma_start(h2[0:64, :], d["h0s"])
        nc.sync.dma_start(h2[64:128, :], d["h0s"])
        words = sbm.tile([BS, E], F32)
        nc.sync.dma_start(words[:], d["words"])
        mask_sb = sbm.tile([BS, ML], F32)
        nc.sync.dma_start(mask_sb[:], d["mask"])
        ctxt = sbm.tile([BS, E], F32)
        h_new = sbm.tile([BS, E], F32)
        c_new = sbm.tile([BS, E], F32)

        # ================= stage 1: attention =================
        with ExitStack() as s1:
            encp = s1.enter_context(tc.tile_pool(name="encp", bufs=5))
            accp = s1.enter_context(tc.tile_pool(name="accp", bufs=2))
            tmp1 = s1.enter_context(tc.tile_pool(name="tmp1", bufs=1))

            enc_c = []
            for c in range(5):
                t = encp.tile([128, 5, E], F32, name=f"enc{c}", tag="enc")
                nc.sync.dma_start(t[:], d["enc"][:, c * 5:(c + 1) * 5, :])
                enc_c.append(t)
            enc_t = [enc_c[l2 // 5][:, l2 % 5, :] for l2 in range(L2)]

            energy2 = tmp1.tile([128, L2], F32)
            scr = tmp1.tile([128, E], F32)
            for l2 in range(L2):
                nc.vector.scalar_tensor_tensor(
                    out=scr[:], in0=enc_t[l2], scalar=1.0, in1=h2[:],
                    op0=MULT, op1=MULT, accum_out=energy2[:, l2:l2 + 1])

            energy = tmp1.tile([BS, ML], F32)
            nc.vector.tensor_copy(energy[:, 0:L2], energy2[0:64, :])
            nc.sync.dma_start(energy[:, L2:ML], energy2[64:128, :])

            # masked softmax (fp32; equals reference softmax+mask+renorm)
            rmax = tmp1.tile([BS, 1], F32)
            nc.vector.tensor_reduce(rmax[:], energy[:],
                                    axis=mybir.AxisListType.X,
                                    op=mybir.AluOpType.max)
            nrmax = tmp1.tile([BS, 1], F32)
            nc.vector.tensor_scalar_mul(nrmax[:], rmax[:], -1.0)
            expd = tmp1.tile([BS, ML], F32)
            nc.scalar.activation(expd[:], energy[:], AF.Exp,
                                 bias=nrmax[:], scale=1.0)
            masked = tmp1.tile([BS, ML], F32)
            nc.vector.tensor_mul(masked[:], expd[:], mask_sb[:])
            ssum = tmp1.tile([BS, 1], F32)
            nc.vector.tensor_reduce(ssum[:], masked[:],
                                    axis=mybir.AxisListType.X, op=ADD)
            rinv = tmp1.tile([BS, 1], F32)
            nc.vector.reciprocal(rinv[:], ssum[:])
            normed = tmp1.tile([BS, ML], F32)
            nc.vector.tensor_scalar_mul(normed[:], masked[:], rinv[:])

            normed2 = tmp1.tile([128, L2], F32)
            nc.vector.tensor_copy(normed2[0:64, :], normed[:, 0:L2])
            nc.sync.dma_start(normed2[64:128, :], normed[:, L2:ML])

            # contexts: chained multiply-accumulate (ping-pong)
            acc_prev = accp.tile([128, E], F32, name="acc0", tag="acc")
            nc.vector.tensor_scalar_mul(acc_prev[:], enc_t[0],
                                        normed2[:, 0:1])
            for l2 in range(1, L2):
                acc_cur = accp.tile([128, E], F32, name=f"acc{l2}", tag="acc")
                nc.vector.scalar_tensor_tensor(
                    out=acc_cur[:], in0=enc_t[l2],
                    scalar=normed2[:, l2:l2 + 1],
                    in1=acc_prev[:], op0=MULT, op1=ADD)
                acc_prev = acc_cur
            ctx_hi = tmp1.tile([BS, E], F32)
            nc.sync.dma_start(ctx_hi[:], acc_prev[64:128, :])
            nc.vector.tensor_add(ctxt[:], acc_prev[0:64, :], ctx_hi[:])

        # ================= gates + LSTM pointwise + stage 2 =================
        with ExitStack() as s2:
            gp = s2.enter_context(tc.tile_pool(name="gp", bufs=1))
            wgp = s2.enter_context(tc.tile_pool(name="wgp", bufs=3))
            sp = s2.enter_context(tc.tile_pool(name="sp", bufs=1))
            wtp = s2.enter_context(tc.tile_pool(name="wtp", bufs=3))
            outp = s2.enter_context(tc.tile_pool(name="outp", bufs=4))

            bproj_sb = sp.tile([1, VS], BF16)
            nc.sync.dma_start(bproj_sb[:], d["bproj"])
            biasg_sb = gp.tile([1, G], BF16)
            nc.sync.dma_start(biasg_sb[:], d["biasg"])

            wordsT = gp.tile([128, KC, BS], BF16)
            ctxT = gp.tile([128, KC, BS], BF16)
            for k in range(KC):
                pt = pst.tile([128, BS], F32, name=f"ptw{k}", tag="pt")
                nc.tensor.transpose(pt[:], words[:, k * 128:(k + 1) * 128],
                                    identity[0:BS, 0:BS])
                nc.scalar.copy(wordsT[:, k, :], pt[:])
                pt2 = pst.tile([128, BS], F32, name=f"ptc{k}", tag="pt")
                nc.tensor.transpose(pt2[:], ctxt[:, k * 128:(k + 1) * 128],
                                    identity[0:BS, 0:BS])
                nc.scalar.copy(ctxT[:, k, :], pt2[:])

            g_act = gp.tile([BS, 8, 512], F32)
            for q in range(4):
                wa = wgp.tile([128, KC, 1024], BF16, name=f"wih{q}", tag="wg")
                nc.sync.dma_start(
                    wa[:], d["wihT"][:, q * 1024:(q + 1) * 1024]
                    .rearrange("(k p) n -> p k n", p=128))
                wb = wgp.tile([128, KC, 1024], BF16, name=f"whh{q}", tag="wg")
                nc.sync.dma_start(
                    wb[:], d["whhT"][:, q * 1024:(q + 1) * 1024]
                    .rearrange("(k p) n -> p k n", p=128))
                for half in range(2):
                    n = q * 2 + half
                    hs = slice(half * 512, (half + 1) * 512)
                    ps = psg.tile([BS, 512], F32, name=f"gps{n}", tag="gps")
                    nc.tensor.matmul(ps[:], ones_bf[:, 0:BS],
                                     biasg_sb[:, n * 512:(n + 1) * 512],
                                     start=True, stop=False)
                    for k in range(KC):
                        nc.tensor.matmul(ps[:], wordsT[:, k, :], wa[:, k, hs],
                                         start=False, stop=False)
                    for k in range(KC):
                        nc.tensor.matmul(ps[:], ctxT[:, k, :], wb[:, k, hs],
                                         start=False, stop=(k == KC - 1))
                    func = AF.Tanh if n in (4, 5) else AF.Sigmoid
                    nc.scalar.activation(g_act[:, n, :], ps[:], func,
                                         bias=zbias[0:BS, :], scale=1.0)

            iv = g_act[:, 0:2, :]
            fv = g_act[:, 2:4, :]
            gv = g_act[:, 4:6, :]
            ov = g_act[:, 6:8, :]
            ctx_v = ctxt[:].rearrange("b (x y) -> b x y", x=2)
            ig = gp.tile([BS, 2, 512], F32)
            nc.vector.tensor_mul(ig[:], iv, gv)
            fc = gp.tile([BS, 2, 512], F32)
            nc.vector.tensor_mul(fc[:], fv, ctx_v)
            cn_v = c_new[:].rearrange("b (x y) -> b x y", x=2)
            nc.vector.tensor_add(cn_v, ig[:], fc[:])
            tanh_c = gp.tile([BS, E], F32)
            nc.scalar.activation(tanh_c[:], c_new[:], AF.Tanh,
                                 bias=zbias[0:BS, :], scale=1.0)
            th_v = tanh_c[:].rearrange("b (x y) -> b x y", x=2)
            hn_v = h_new[:].rearrange("b (x y) -> b x y", x=2)
            nc.vector.tensor_mul(hn_v, ov, th_v)
            nc.sync.dma_start(d["c_new_s"], c_new[:])
            nc.sync.dma_start(d["h_new_s"], h_new[:])

            # h_new^T (bf16) -> DRAM bounce
            hT = gp.tile([128, KC, BS], BF16)
            for k in range(KC):
                pt = pst.tile([128, BS], F32, name=f"pth{k}", tag="pt")
                nc.tensor.transpose(pt[:], h_new[:, k * 128:(k + 1) * 128],
                                    identity[0:BS, 0:BS])
                nc.scalar.copy(hT[:, k, :], pt[:])
            bounce = dram.tile([E, BS], BF16)
            nc.sync.dma_start(bounce[:].rearrange("(k p) b -> p k b", p=128),
                              hT[:])

            # stage-2 weight tiles: one 2MB DMA per 1024-col chunk (emitted
            # after the gates weight DMAs so the in-order queue cannot cycle)
            wt_wide = []
            for n2 in range((VS + 1023) // 1024):
                c0 = n2 * 1024
                cw = min(1024, VS - c0)
                wt = wtp.tile([128, KC, 1024], BF16, name=f"wt{n2}", tag="wt")
                nc.sync.dma_start(wt[:, :, 0:cw],
                                  d["wT"][:, c0:c0 + cw]
                                  .rearrange("(k p) n -> p k n", p=128))
                wt_wide.append(wt)

            gathered = dram.tile([NCORES, E, BS], BF16, addr_space="Shared")
            nc.gpsimd.collective_compute(
                "AllGather", mybir.AluOpType.bypass,
                replica_groups=[list(range(NCORES))],
                ins=[bounce[:].opt()], outs=[gathered[:].opt()])

            # ================= stage 2: vocab projection =================
            HT = sp.tile([128, 4, KC, 128], BF16)
            for j in range(4):
                for hc in range(2):
                    srcc = gathered[2 * j + hc, :, :]
                    dstc = HT[:, j, :, hc * 64:(hc + 1) * 64]
                    nc.sync.dma_start(
                        dstc, srcc.rearrange("(k p) b -> p k b", p=128))

            for n in range(NT):
                n0 = n * 512
                nw = min(512, VS - n0)
                wtile = wt_wide[n // 2]
                ws = slice((n % 2) * 512, (n % 2) * 512 + nw)
                for j in range(4):
                    ps = pso.tile([128, 512], F32, name=f"ops{n}_{j}", tag="ops")
                    nc.tensor.matmul(ps[:, 0:nw], ones_bf[:],
                                     bproj_sb[:, n0:n0 + nw],
                                     start=True, stop=False)
                    for k in range(KC):
                        nc.tensor.matmul(ps[:, 0:nw], HT[:, j, k, :],
                                         wtile[:, k, ws],
                                         start=False, stop=(k == KC - 1))
                    ot = outp.tile([128, 512], F32, name=f"ot{n}_{j}", tag="ot")
                    nc.scalar.copy(ot[:, 0:nw], ps[:, 0:nw])
                    nc.sync.dma_start(
                        d["logits_s"][j * 128:(j + 1) * 128, n0:n0 + nw],
                        ot[:, 0:nw])


def _build():
    nc = bacc.Bacc("TRN2", num_devices=NCORES)
    d = {}

    def inp(name, shape, dt):
        d[name] = nc.dram_tensor(name, shape, dt, kind="ExternalInput").ap()

    def outp(name, shape, dt):
        d[name] = nc.dram_tensor(name, shape, dt, kind="ExternalOutput").ap()

    inp("enc", [128, L2, E], F32)
    inp("h0s", [BS, E], F32)
    inp("words", [BS, E], F32)
    inp("mask", [BS, ML], F32)
    inp("wihT", [E, G], BF16)
    inp("whhT", [E, G], BF16)
    inp("biasg", [1, G], BF16)
    inp("wT", [E, VS], BF16)
    inp("bproj", [1, VS], BF16)
    outp("logits_s", [B, VS], F32)
    outp("h_new_s", [BS, E], F32)
    outp("c_new_s", [BS, E], F32)

    with tile.TileContext(nc) as tc:
        _body(nc, tc, d)
    nc.compile()
    return nc


def make_in_maps(target_sentences, encoder_outputs, h0, padded_positions,
                 emb, w_ih, w_hh, b_ih, b_hh, w_proj, b_proj):
    bf = ml_dtypes.bfloat16
    target_sentences = np.asarray(target_sentences)
    encoder_outputs = np.ascontiguousarray(
        np.asarray(encoder_outputs, np.float32))
    h00 = np.asarray(h0, np.float32)[0]
    padded = np.asarray(padded_positions)
    emb = np.asarray(emb, np.float32)
    w_proj = np.asarray(w_proj, np.float32)
    b_proj = np.asarray(b_proj, np.float32)

    words_all = emb[target_sentences]
    mask_all = 1.0 - padded.astype(np.float32)
    wihT = np.ascontiguousarray(np.asarray(w_ih, np.float32).T).astype(bf)
    whhT = np.ascontiguousarray(np.asarray(w_hh, np.float32).T).astype(bf)
    biasg = (np.asarray(b_ih, np.float32)
             + np.asarray(b_hh, np.float32))[None, :].astype(bf)
    wTf = np.ascontiguousarray(w_proj.T)

    in_maps = []
    for c in range(NCORES):
        bs = slice(c * BS, (c + 1) * BS)
        vs = slice(c * VS, (c + 1) * VS)
        enc_s = encoder_outputs[bs]
        encr = np.ascontiguousarray(
            enc_s.reshape(BS, 2, L2, E).transpose(1, 0, 2, 3).reshape(128, L2, E))
        in_maps.append({
            "enc": encr,
            "h0s": np.ascontiguousarray(h00[bs]),
            "words": np.ascontiguousarray(words_all[bs]),
            "mask": np.ascontiguousarray(mask_all[bs]),
            "wihT": wihT,
            "whhT": whhT,
            "biasg": biasg,
            "wT": np.ascontiguousarray(wTf[:, vs]).astype(bf),
            "bproj": np.ascontiguousarray(b_proj[vs])[None, :].astype(bf),
        })
    return in_maps


def kernel(target_sentences, encoder_outputs, h0, c0, padded_positions,
           max_len, emb, w_ih, w_hh, b_ih, b_hh, w_proj, b_proj):
    if "nc" not in _CACHE:
        _CACHE["nc"] = _build()
    nc = _CACHE["nc"]

    in_maps = make_in_maps(target_sentences, encoder_outputs, h0,
                           padded_positions, emb, w_ih, w_hh, b_ih, b_hh,
                           w_proj, b_proj)
    res = bass_utils.run_bass_kernel_spmd(nc, in_maps,
                                          core_ids=list(range(NCORES)))
    rs = res.results
    logits = np.concatenate([rs[c]["logits_s"] for c in range(NCORES)], axis=1)
    h_new = np.concatenate([rs[c]["h_new_s"] for c in range(NCORES)],
                           axis=0)[None]
    c_new = np.concatenate([rs[c]["c_new_s"] for c in range(NCORES)],
                           axis=0)[None]
    return logits, h_new, c_new
